# revision 30
# baseline (speedup 1.0000x reference)
"""Multi-head causal attention (B=4, S=2048, D=1024, H=16) on 8 TRN2 cores.

Sharding: core c = (batch b = c//2, head-group g = c%2). Each core computes
8 heads of one batch end-to-end: QKV projections, causal flash attention,
and its half of the output projection (row-parallel Wo). Host sums the two
partial outputs per batch (the "all-reduce"); bias is added on device,
split half per core. Output partials are bf16 (summed in fp32 on host).

Device dataflow is fully transposed (xT in, outT out) so no on-device
transposes of activations are needed; V is transposed via one xbar DMA
transpose per pair (into a partition-tiled [128, NKB, 128] layout) plus
cheap DVE re-copies that insert the denominator ones column. All matmuls
are bf16 (fp32 PSUM accumulation) except a tiny f32r matmul that
broadcasts softmax reciprocals across partitions. Scores for the two
heads of a pair are issued back-to-back into disjoint PE row groups so
they run concurrently (contraction is only 64 deep). The causal structure
skips invalid 128x512 blocks entirely and trims the invalid left columns
of diagonal blocks from the scores/exp/mask/PV chain.

Scheduling: the next pair's QKV projection matmuls (and, on the last
pair, the output-projection matmuls) are pumped into the Tensor queue
between attention blocks so the PE stays busy while the Scalar engine
works through the exp chain. Softmax normalization happens per q-block
(reading the PV accumulator directly from PSUM) instead of per pair.
Input/weight DMAs are split across both HWDGE rings (sync + scalar) and
ordered so the first matmul can start as early as possible.
"""
import os
import sys
import types

import numpy as np
import ml_dtypes

from concourse import bacc, tile, bass_utils, mybir

B, S, D, H = 4, 2048, 1024, 16
HD = 64            # head dim
G = 2              # head groups (cores per batch)
DG = D // G        # 512 cols per core
NP = DG // 128     # 4 head-pairs per core
NCH = D // 128     # 8 contraction chunks
SB = 512           # q block
NSB = S // SB      # 4 q blocks
NKB = S // 128     # 16 k blocks

f32 = mybir.dt.float32
f32r = mybir.dt.float32r
bf16 = mybir.dt.bfloat16

LAST_RESULTS = None
_CACHE = {}


def _install_trace_shim():
    """Register the axon NTFF profile hook if this image's antenv lacks it."""
    if "antenv.axon_hooks" in sys.modules:
        return
    try:
        from trn_agent_boot.trn_boot import _ntff_profile_via_ctypes

        hook = _ntff_profile_via_ctypes("/opt/axon/libaxon_pjrt.so")
        mod = types.ModuleType("antenv.axon_hooks")
        mod.get_axon_ntff_profile_hook = lambda: hook
        mod.set_axon_ntff_profile_hook = lambda h: None
        sys.modules["antenv.axon_hooks"] = mod
        import antenv

        antenv.axon_hooks = mod
    except Exception:
        pass


def _build_program():
    nc = bacc.Bacc("TRN2", target_bir_lowering=False, debug=False)

    xT_d = nc.dram_tensor("xT", [D, S], bf16, kind="ExternalInput").ap()
    wq_d = nc.dram_tensor("Wq", [NP, 128, NCH, 128], bf16, kind="ExternalInput").ap()
    wk_d = nc.dram_tensor("Wk", [NP, 128, NCH, 128], bf16, kind="ExternalInput").ap()
    wv_d = nc.dram_tensor("Wv", [NP, 128, NCH, 128], bf16, kind="ExternalInput").ap()
    wo_d = nc.dram_tensor("Wo", [128, NP, NCH, 128], bf16, kind="ExternalInput").ap()
    bo_d = nc.dram_tensor("bo2", [128, NCH], f32, kind="ExternalInput").ap()
    mask_d = nc.dram_tensor("masks", [128, 4, SB], bf16, kind="ExternalInput").ap()
    out_d = nc.dram_tensor("outT", [D, S], bf16, kind="ExternalOutput").ap()

    xT_src = xT_d.rearrange("(c k) s -> k c s", k=128)
    out_dst = out_d.rearrange("(c k) s -> k c s", k=128)

    with tile.TileContext(nc) as tc:
        with (
            tc.tile_pool(name="const", bufs=1) as constp,
            tc.tile_pool(name="psc", bufs=2, space="PSUM") as psc,
            tc.tile_pool(name="psq", bufs=2, space="PSUM") as psq,
            tc.tile_pool(name="psy", bufs=2, space="PSUM") as psy,
            tc.tile_pool(name="ynormp", bufs=4) as ynormp,
            tc.tile_pool(name="outp", bufs=3) as outp,
        ):
            mask_sb = constp.tile([128, 4, SB], bf16)
            bo_sb = constp.tile([128, NCH], f32)
            wo_sb = constp.tile([128, NP, NCH, 128], bf16)
            # scalar HWDGE ring: tiny consts now; masks/wo are issued
            # after the xt quarters so they never delay the first matmuls.
            nc.scalar.dma_start(bo_sb[:], bo_d[:])

            ynorm = []  # per-pair [128, S] bf16 normalized attention outputs

            with (
                tc.tile_pool(name="xtp", bufs=1) as xtp,
                tc.tile_pool(name="wp", bufs=3) as wp,
                tc.tile_pool(name="qkv", bufs=2) as qkvp,
                tc.tile_pool(name="vtp", bufs=2) as vtp,
                tc.tile_pool(name="vstp", bufs=2) as vstp,
                tc.tile_pool(name="vp", bufs=2) as vpool,
                tc.tile_pool(name="pp", bufs=4) as ppool,
                tc.tile_pool(name="yun", bufs=4) as yunp,
                tc.tile_pool(name="rp", bufs=2) as rpool,
                tc.tile_pool(name="rdp", bufs=4, space="DRAM") as rdpool,
            ):
                xt = xtp.tile([128, NCH, S], bf16)

                def alloc_pair(pp_):
                    qt_ = qkvp.tile([128, S], bf16, tag="qt", name="qt")
                    kt_ = qkvp.tile([128, S], bf16, tag="kt", name="kt")
                    vt_ = vtp.tile([128, S], bf16, tag="vt", name="vt")
                    # per (kb, h): 64 head dims + ones col at 64 (+1 pad
                    # so the h stride is 4B-aligned for fast DVE copies)
                    vs_ = vpool.tile([128, NKB, 2, 66], bf16, tag="v",
                                     name="v_sb")
                    vst_ = vstp.tile([128, NKB, 128], bf16, tag="vst",
                                     name="vst")
                    nc.vector.memset(vs_[:, :, :, 64:65], 1.0)
                    return qt_, kt_, vt_, vs_, vst_

                def prefetch_w(pp_):
                    tiles = []
                    for nm, wd in (("wv", wv_d), ("wq", wq_d), ("wk", wk_d)):
                        wt = wp.tile([128, NCH, 128], bf16, tag=nm, name=nm)
                        nc.sync.dma_start(wt[:], wd[pp_])
                        tiles.append(wt)
                    return tiles

                def emit_qkv(pp_, w_tiles, qt_, kt_, vt_, vs_, vst_,
                             half_major=False):
                    """Yield small units of pair pp_'s QKV projection work.

                    V is projected first so its transpose (one xbar DMA
                    for the whole [128, S] tile) can start early. Weight
                    tiles were DMA'd well in advance by prefetch_w.
                    half_major=True consumes the input in S-halves (for
                    the prologue pair, paced by the xt quarter DMAs).
                    """
                    yield
                    halves = ((0, 1),) if not half_major else ((0,), (1,))
                    for hgrp in halves:
                      for half in hgrp:
                        for wi, (wt, dst) in enumerate(
                                zip(w_tiles, (vt_, qt_, kt_))):
                            if half == 1 and not half_major:
                                continue
                            for sblk in ((2 * half, 2 * half + 1)
                                         if half_major else (0, 1, 2, 3)):
                                acc = psq.tile([128, SB], f32, tag="acc",
                                               name="qacc")
                                for ci in range(NCH):
                                    nc.tensor.matmul(
                                        acc[:],
                                        wt[:, ci, :],
                                        xt[:, ci,
                                           sblk * SB:(sblk + 1) * SB],
                                        start=(ci == 0),
                                        stop=(ci == NCH - 1),
                                    )
                                    if ci % 4 == 3:
                                        yield
                                nc.vector.tensor_copy(
                                    dst[:, sblk * SB:(sblk + 1) * SB],
                                    acc[:],
                                )
                                yield
                            if wi == 0 and (half == 1 or not half_major):
                                # vt complete: one xbar transpose VT -> V
                                # (s = kb*128 + partition tiling), then
                                # per-kb DVE copies inserting the ones col.
                                nc.sync.dma_start_transpose(vst_[:], vt_[:])
                                for kb in range(NKB):
                                    nc.vector.tensor_copy(
                                        vs_[:, kb, :, 0:64],
                                        vst_[:, kb, :].rearrange(
                                            "k (h d) -> k h d", h=2),
                                    )
                                    if kb % 4 == 3:
                                        yield

                def emit_outproj(j, g2s=(0, 1, 2, 3)):
                    """outT[:, j block] = Wo_g.T @ ynorm (+ bo/2), bf16."""
                    for g2 in g2s:
                        ot = outp.tile([128, 2, SB], bf16, tag="ot",
                                       name="ot")
                        for si in range(2):
                            dc = 2 * g2 + si
                            acc = psq.tile([128, SB], f32, tag="acc",
                                           name="oacc")
                            for pp in range(NP):
                                nc.tensor.matmul(
                                    acc[:],
                                    wo_sb[:, pp, dc, :],
                                    ynorm[pp][:, j * SB:(j + 1) * SB],
                                    start=(pp == 0),
                                    stop=(pp == NP - 1),
                                )
                                if pp % 2 == 1:
                                    yield
                            nc.vector.tensor_scalar_add(
                                ot[:, si, :],
                                acc[:],
                                bo_sb[:, dc:dc + 1],
                            )
                        nc.sync.dma_start(
                            out_dst[:, 2 * g2:2 * g2 + 2,
                                    j * SB:(j + 1) * SB],
                            ot[:],
                        )
                        yield

                pending = []

                def pump(n=1):
                    for _ in range(n):
                        while pending:
                            try:
                                next(pending[0])
                                break
                            except StopIteration:
                                pending.pop(0)
                        else:
                            break

                def drain():
                    while pending:
                        try:
                            next(pending[0])
                        except StopIteration:
                            pending.pop(0)

                # prologue: pair-0 weights on the sync ring first, then x
                wts = {0: prefetch_w(0)}
                cur = alloc_pair(0)
                for qtr in range(4):
                    # split each quarter across both HWDGE rings so the
                    # earliest-needed slices finish ~2x sooner
                    nc.sync.dma_start(
                        xt[:, 0:4, qtr * SB:(qtr + 1) * SB],
                        xT_src[:, 0:4, qtr * SB:(qtr + 1) * SB],
                    )
                    nc.scalar.dma_start(
                        xt[:, 4:8, qtr * SB:(qtr + 1) * SB],
                        xT_src[:, 4:8, qtr * SB:(qtr + 1) * SB],
                    )
                nc.sync.dma_start(mask_sb[:], mask_d[:])
                nc.sync.dma_start(wo_sb[:], wo_d[:])
                # HAM warmup: ~4us of dummy matmuls un-throttle the PE clock
                # (1.2 -> 2.4 GHz) while the input DMAs are still in flight
                warm_in = rpool.tile([128, 128], bf16, tag="wi", name="warm_in")
                nc.vector.memset(warm_in[:], 0.0)
                warm = psq.tile([128, SB], f32, tag="acc", name="warm")
                for _ in range(40):
                    nc.tensor.matmul(warm[:, 0:64], warm_in[:],
                                     warm_in[:, 0:64], start=True, stop=True)
                warm_rd = rpool.tile([1, SB], f32, tag="wr", name="warm_rd")
                nc.vector.tensor_copy(warm_rd[0:1, 0:64], warm[0:1, 0:64])
                for _ in emit_qkv(0, wts[0], *cur, half_major=True):
                    pass
                wts[1] = prefetch_w(1)

                def attn_j(p, j, qt, kt, vs_, yn):
                    """One q-block of causal attention for pair p.

                    Score blocks are processed two at a time so the four
                    row-tiled score matmuls pipeline with full 2-head
                    concurrency; PV lags one block-pair behind.
                    """
                    if True:
                        nkb_j = 4 * (j + 1)
                        yaccs = [
                            psy.tile([128, SB], f32, tag="y", name="yacc")
                            for _ in range(2)
                        ]
                        pend = []
                        for ki in range(nkb_j // 2):
                            newp = []
                            for kb in (2 * ki, 2 * ki + 1):
                                d = kb - 4 * j
                                qlo = max(0, d) * 128  # causal column trim
                                sc = psc.tile([128, 2, SB], f32, tag="sc",
                                              name="sc")
                                for h in range(2):
                                    hlo, hhi = h * 64, (h + 1) * 64
                                    nc.tensor.matmul(
                                        sc[:, h, qlo:],
                                        kt[hlo:hhi, kb * 128:(kb + 1) * 128],
                                        qt[hlo:hhi,
                                           j * SB + qlo:(j + 1) * SB],
                                        start=True,
                                        stop=True,
                                    )
                                newp.append((kb, sc, qlo))
                            for bi, (kb, sc, qlo) in enumerate(newp):
                                d = kb - 4 * j
                                pt = ppool.tile([128, 2, SB], bf16, tag="p",
                                                name="pt")
                                nc.scalar.activation(
                                    pt[:, :, qlo:],
                                    sc[:, :, qlo:],
                                    mybir.ActivationFunctionType.Exp,
                                    scale=0.125,
                                )
                                if d >= 0:
                                    for h in range(2):
                                        nc.vector.tensor_mul(
                                            pt[:, h, qlo:],
                                            pt[:, h, qlo:],
                                            mask_sb[:, d, qlo:],
                                        )
                                newp[bi] = (kb, pt, qlo)
                            for kb_, pt_, qlo_ in pend:
                                for h in range(2):
                                    nc.tensor.matmul(
                                        yaccs[h][0:65, qlo_:],
                                        vs_[:, kb_, h, 0:65],
                                        pt_[:, h, qlo_:],
                                        start=(kb_ == 0),
                                        stop=False,
                                    )
                            pend = newp
                            pump(3 if p == NP - 1 else 2)
                        for kb_, pt_, qlo_ in pend:
                            for h in range(2):
                                nc.tensor.matmul(
                                    yaccs[h][0:65, qlo_:],
                                    vs_[:, kb_, h, 0:65],
                                    pt_[:, h, qlo_:],
                                    start=(kb_ == 0),
                                    stop=(kb_ == nkb_j - 1),
                                )
                        # ---- per-j normalize: yn = yacc[0:64] / yacc[64].
                        # The reciprocal row is broadcast across partitions
                        # by one DMA with a stride-0 free dim.
                        last = (p == NP - 1 and j == NSB - 1)
                        y_uns = []
                        rts = []
                        for h in range(2):
                            if not last:
                                y_un = yunp.tile([65, SB], f32, tag="yun",
                                                 name="y_un")
                                nc.vector.tensor_copy(y_un[:],
                                                      yaccs[h][0:65, :])
                                den = y_un[64:65, :]
                            else:
                                # final q-block: no need to free PSUM fast;
                                # read yacc directly to shorten the chain
                                y_un = yaccs[h]
                                den = yaccs[h][64:65, :]
                            rt = rpool.tile([1, SB], f32, tag="r",
                                            name="rt")
                            # custom-DVE recip only works at base
                            # partition 0: cross-partition copy first
                            nc.vector.tensor_copy(rt[:], den)
                            nc.vector.reciprocal_approx_fast(rt[:], rt[:])
                            y_uns.append(y_un)
                            rts.append(rt)
                        rbs = []
                        for h in range(2):
                            rsc = rdpool.tile([1, SB], f32, tag="rsc",
                                              name="rsc")
                            nc.sync.dma_start(rsc[:], rts[h][:])
                            rb = rpool.tile([64, SB], f32, tag="rb",
                                            name="rb")
                            nc.sync.dma_start(
                                rb[:], rsc.to_broadcast([64, SB]))
                            rbs.append(rb)
                        # (write h, read h) stay adjacent per head: a write
                        # whose recip isn't ready must not block the other
                        # head's broadcast read in the ring FIFO
                        for h in range(2):
                            hlo, hhi = h * 64, (h + 1) * 64
                            nc.vector.tensor_mul(
                                yn[hlo:hhi, j * SB:(j + 1) * SB],
                                y_uns[h][0:64, :],
                                rbs[h][:],
                            )
                        pump()

                for p in range(NP):
                    qt, kt, vt, vs_, vst_ = cur
                    if p + 2 < NP:
                        wts[p + 2] = prefetch_w(p + 2)
                    if p + 1 < NP:
                        nxt = alloc_pair(p + 1)
                        pending.append(emit_qkv(p + 1, wts[p + 1], *nxt))
                    else:
                        nxt = None
                    yn = ynormp.tile([128, S], bf16, tag="yn", name="yn")
                    ynorm.append(yn)
                    for j in range(NSB):
                        attn_j(p, j, qt, kt, vs_, yn)
                        if p == NP - 1:
                            if j == 2:
                                # hold back half of j=2's output projection:
                                # it has no dependency on the last q-block's
                                # softmax, so it fills the PE while the
                                # final normalize chain completes
                                pending.append(emit_outproj(2, (0, 1)))
                            elif j == 3:
                                pending.append(emit_outproj(2, (2, 3)))
                                pending.append(emit_outproj(3))
                            else:
                                pending.append(emit_outproj(j))
                            pump()
                    drain()
                    cur = nxt

    nc.compile()
    return nc


def _get_program():
    if "nc" not in _CACHE:
        _CACHE["nc"] = _build_program()
    return _CACHE["nc"]


def kernel(x, Wq, Wk, Wv, Wo, bo):
    global LAST_RESULTS
    x = np.asarray(x, dtype=np.float32)
    Wq = np.asarray(Wq, dtype=np.float32)
    Wk = np.asarray(Wk, dtype=np.float32)
    Wv = np.asarray(Wv, dtype=np.float32)
    Wo = np.asarray(Wo, dtype=np.float32)
    bo = np.asarray(bo, dtype=np.float32)

    nc = _get_program()

    # constants shared by all cores
    masks = np.zeros((128, 4, SB), dtype=ml_dtypes.bfloat16)
    kk = np.arange(128)[:, None]
    qq = np.arange(SB)[None, :]
    for d in range(4):
        masks[:, d, :] = (128 * d + kk <= qq).astype(ml_dtypes.bfloat16)
    bo2 = np.ascontiguousarray((bo / 2.0).reshape(NCH, 128).T)

    def wshape(w):  # [D, DG] -> [NP, 128, NCH, 128] bf16
        return np.ascontiguousarray(
            w.reshape(NCH, 128, NP, 128).transpose(2, 1, 0, 3)
        ).astype(ml_dtypes.bfloat16)

    in_maps = []
    for c in range(8):
        b, g = c // 2, c % 2
        xT = np.ascontiguousarray(x[b].T).astype(ml_dtypes.bfloat16)
        wo_g = Wo[g * DG:(g + 1) * DG, :]
        wo_dev = np.ascontiguousarray(
            wo_g.reshape(NP, 128, NCH, 128).transpose(1, 0, 2, 3)
        ).astype(ml_dtypes.bfloat16)
        in_maps.append({
            "xT": xT,
            "Wq": wshape(Wq[:, g * DG:(g + 1) * DG]),
            "Wk": wshape(Wk[:, g * DG:(g + 1) * DG]),
            "Wv": wshape(Wv[:, g * DG:(g + 1) * DG]),
            "Wo": wo_dev,
            "bo2": bo2,
            "masks": masks,
        })

    trace = bool(os.environ.get("BASS_TRACE"))
    if trace:
        _install_trace_shim()
    res = None
    for attempt in range(3):
        try:
            res = bass_utils.run_bass_kernel_spmd(
                nc, in_maps, core_ids=list(range(8)), trace=trace)
            break
        except Exception:
            if attempt == 2:
                raise
    LAST_RESULTS = res

    out = np.empty((B, S, D), dtype=np.float32)
    for b in range(B):
        acc = (res.results[2 * b]["outT"].astype(np.float32)
               + res.results[2 * b + 1]["outT"].astype(np.float32))
        out[b] = acc.T
    return out


# revision 31
# speedup vs baseline: 1.0152x; 1.0152x over previous
"""Multi-head causal attention (B=4, S=2048, D=1024, H=16) on 8 TRN2 cores.

Sharding: core c = (batch b = c//2, head-group g = c%2). Each core computes
8 heads of one batch end-to-end: QKV projections, causal flash attention,
and its half of the output projection (row-parallel Wo). Host sums the two
partial outputs per batch (the "all-reduce"); bias is added on device,
split half per core. Output partials are bf16 (summed in fp32 on host).

Device dataflow is fully transposed (xT in, outT out) so no on-device
transposes of activations are needed; V is transposed via one xbar DMA
transpose per pair (into a partition-tiled [128, NKB, 128] layout) plus
cheap DVE re-copies that insert the denominator ones column. All matmuls
are bf16 (fp32 PSUM accumulation) except a tiny f32r matmul that
broadcasts softmax reciprocals across partitions. Scores for the two
heads of a pair are issued back-to-back into disjoint PE row groups so
they run concurrently (contraction is only 64 deep). The causal structure
skips invalid 128x512 blocks entirely and trims the invalid left columns
of diagonal blocks from the scores/exp/mask/PV chain.

Scheduling: the next pair's QKV projection matmuls (and, on the last
pair, the output-projection matmuls) are pumped into the Tensor queue
between attention blocks so the PE stays busy while the Scalar engine
works through the exp chain. Softmax normalization happens per q-block
(reading the PV accumulator directly from PSUM) instead of per pair.
Input/weight DMAs are split across both HWDGE rings (sync + scalar) and
ordered so the first matmul can start as early as possible.
"""
import os
import sys
import types

import numpy as np
import ml_dtypes

from concourse import bacc, tile, bass_utils, mybir

B, S, D, H = 4, 2048, 1024, 16
HD = 64            # head dim
G = 2              # head groups (cores per batch)
DG = D // G        # 512 cols per core
NP = DG // 128     # 4 head-pairs per core
NCH = D // 128     # 8 contraction chunks
SB = 512           # q block
NSB = S // SB      # 4 q blocks
NKB = S // 128     # 16 k blocks

f32 = mybir.dt.float32
f32r = mybir.dt.float32r
bf16 = mybir.dt.bfloat16

LAST_RESULTS = None
_CACHE = {}


def _install_trace_shim():
    """Register the axon NTFF profile hook if this image's antenv lacks it."""
    if "antenv.axon_hooks" in sys.modules:
        return
    try:
        from trn_agent_boot.trn_boot import _ntff_profile_via_ctypes

        hook = _ntff_profile_via_ctypes("/opt/axon/libaxon_pjrt.so")
        mod = types.ModuleType("antenv.axon_hooks")
        mod.get_axon_ntff_profile_hook = lambda: hook
        mod.set_axon_ntff_profile_hook = lambda h: None
        sys.modules["antenv.axon_hooks"] = mod
        import antenv

        antenv.axon_hooks = mod
    except Exception:
        pass


def _build_program():
    nc = bacc.Bacc("TRN2", target_bir_lowering=False, debug=False)

    xT_d = nc.dram_tensor("xT", [D, S], bf16, kind="ExternalInput").ap()
    wq_d = nc.dram_tensor("Wq", [NP, 128, NCH, 128], bf16, kind="ExternalInput").ap()
    wk_d = nc.dram_tensor("Wk", [NP, 128, NCH, 128], bf16, kind="ExternalInput").ap()
    wv_d = nc.dram_tensor("Wv", [NP, 128, NCH, 128], bf16, kind="ExternalInput").ap()
    wo_d = nc.dram_tensor("Wo", [128, NP, NCH, 128], bf16, kind="ExternalInput").ap()
    bo_d = nc.dram_tensor("bo2", [128, NCH], f32, kind="ExternalInput").ap()
    mask_d = nc.dram_tensor("masks", [128, 4, SB], bf16, kind="ExternalInput").ap()
    out_d = nc.dram_tensor("outT", [D, S], bf16, kind="ExternalOutput").ap()

    xT_src = xT_d.rearrange("(c k) s -> k c s", k=128)
    out_dst = out_d.rearrange("(c k) s -> k c s", k=128)

    with tile.TileContext(nc) as tc:
        with (
            tc.tile_pool(name="const", bufs=1) as constp,
            tc.tile_pool(name="psc", bufs=2, space="PSUM") as psc,
            tc.tile_pool(name="psq", bufs=2, space="PSUM") as psq,
            tc.tile_pool(name="psy", bufs=2, space="PSUM") as psy,
            tc.tile_pool(name="ynormp", bufs=4) as ynormp,
            tc.tile_pool(name="outp", bufs=3) as outp,
        ):
            mask_sb = constp.tile([128, 4, SB], bf16)
            bo_sb = constp.tile([128, NCH], f32)
            wo_sb = constp.tile([128, NP, NCH, 128], bf16)
            # scalar HWDGE ring: tiny consts now; masks/wo are issued
            # after the xt quarters so they never delay the first matmuls.
            nc.scalar.dma_start(bo_sb[:], bo_d[:])

            ynorm = []  # per-pair [128, S] bf16 normalized attention outputs

            with (
                tc.tile_pool(name="xtp", bufs=1) as xtp,
                tc.tile_pool(name="wp", bufs=3) as wp,
                tc.tile_pool(name="qkv", bufs=2) as qkvp,
                tc.tile_pool(name="vtp", bufs=2) as vtp,
                tc.tile_pool(name="vstp", bufs=2) as vstp,
                tc.tile_pool(name="vp", bufs=2) as vpool,
                tc.tile_pool(name="pp", bufs=4) as ppool,
                tc.tile_pool(name="yun", bufs=4) as yunp,
                tc.tile_pool(name="rp", bufs=2) as rpool,
                tc.tile_pool(name="rdp", bufs=4, space="DRAM") as rdpool,
            ):
                xt = xtp.tile([128, NCH, S], bf16)

                def alloc_pair(pp_):
                    qt_ = qkvp.tile([128, S], bf16, tag="qt", name="qt")
                    kt_ = qkvp.tile([128, S], bf16, tag="kt", name="kt")
                    vt_ = vtp.tile([128, S], bf16, tag="vt", name="vt")
                    # per (kb, h): 64 head dims + ones col at 64 (+1 pad
                    # so the h stride is 4B-aligned for fast DVE copies)
                    vs_ = vpool.tile([128, NKB, 2, 66], bf16, tag="v",
                                     name="v_sb")
                    vst_ = vstp.tile([128, NKB, 128], bf16, tag="vst",
                                     name="vst")
                    nc.vector.memset(vs_[:, :, :, 64:65], 1.0)
                    return qt_, kt_, vt_, vs_, vst_

                def prefetch_w(pp_):
                    tiles = []
                    for nm, wd in (("wv", wv_d), ("wq", wq_d), ("wk", wk_d)):
                        wt = wp.tile([128, NCH, 128], bf16, tag=nm, name=nm)
                        nc.sync.dma_start(wt[:], wd[pp_])
                        tiles.append(wt)
                    return tiles

                def emit_qkv(pp_, w_tiles, qt_, kt_, vt_, vs_, vst_,
                             half_major=False):
                    """Yield small units of pair pp_'s QKV projection work.

                    V is projected first so its transpose (one xbar DMA
                    for the whole [128, S] tile) can start early. Weight
                    tiles were DMA'd well in advance by prefetch_w.
                    half_major=True consumes the input in S-halves (for
                    the prologue pair, paced by the xt quarter DMAs).
                    """
                    yield
                    halves = ((0, 1),) if not half_major else ((0,), (1,))
                    for hgrp in halves:
                      for half in hgrp:
                        for wi, (wt, dst) in enumerate(
                                zip(w_tiles, (vt_, qt_, kt_))):
                            if half == 1 and not half_major:
                                continue
                            for sblk in ((2 * half, 2 * half + 1)
                                         if half_major else (0, 1, 2, 3)):
                                acc = psq.tile([128, SB], f32, tag="acc",
                                               name="qacc")
                                for ci in range(NCH):
                                    nc.tensor.matmul(
                                        acc[:],
                                        wt[:, ci, :],
                                        xt[:, ci,
                                           sblk * SB:(sblk + 1) * SB],
                                        start=(ci == 0),
                                        stop=(ci == NCH - 1),
                                    )
                                    if ci % 4 == 3:
                                        yield
                                nc.vector.tensor_copy(
                                    dst[:, sblk * SB:(sblk + 1) * SB],
                                    acc[:],
                                )
                                yield
                            if wi == 0 and (half == 1 or not half_major):
                                # vt complete: one xbar transpose VT -> V
                                # (s = kb*128 + partition tiling), then
                                # per-kb DVE copies inserting the ones col.
                                nc.sync.dma_start_transpose(vst_[:], vt_[:])
                                for kb in range(NKB):
                                    nc.vector.tensor_copy(
                                        vs_[:, kb, :, 0:64],
                                        vst_[:, kb, :].rearrange(
                                            "k (h d) -> k h d", h=2),
                                    )
                                    if kb % 4 == 3:
                                        yield

                def emit_outproj(j, g2s=(0, 1, 2, 3)):
                    """outT[:, j block] = Wo_g.T @ ynorm (+ bo/2), bf16."""
                    for g2 in g2s:
                        ot = outp.tile([128, 2, SB], bf16, tag="ot",
                                       name="ot")
                        for si in range(2):
                            dc = 2 * g2 + si
                            acc = psq.tile([128, SB], f32, tag="acc",
                                           name="oacc")
                            for pp in range(NP):
                                nc.tensor.matmul(
                                    acc[:],
                                    wo_sb[:, pp, dc, :],
                                    ynorm[pp][:, j * SB:(j + 1) * SB],
                                    start=(pp == 0),
                                    stop=(pp == NP - 1),
                                )
                                if pp % 2 == 1:
                                    yield
                            nc.vector.tensor_scalar_add(
                                ot[:, si, :],
                                acc[:],
                                bo_sb[:, dc:dc + 1],
                            )
                        nc.sync.dma_start(
                            out_dst[:, 2 * g2:2 * g2 + 2,
                                    j * SB:(j + 1) * SB],
                            ot[:],
                        )
                        yield

                pending = []

                def pump(n=1):
                    for _ in range(n):
                        while pending:
                            try:
                                next(pending[0])
                                break
                            except StopIteration:
                                pending.pop(0)
                        else:
                            break

                def drain():
                    while pending:
                        try:
                            next(pending[0])
                        except StopIteration:
                            pending.pop(0)

                # prologue: pair-0 weights on the sync ring first, then x
                wts = {0: prefetch_w(0)}
                cur = alloc_pair(0)
                for qtr in range(4):
                    nc.sync.dma_start(
                        xt[:, :, qtr * SB:(qtr + 1) * SB],
                        xT_src[:, :, qtr * SB:(qtr + 1) * SB],
                    )
                nc.sync.dma_start(mask_sb[:], mask_d[:])
                nc.sync.dma_start(wo_sb[:], wo_d[:])
                # HAM warmup: ~4us of dummy matmuls un-throttle the PE clock
                # (1.2 -> 2.4 GHz) while the input DMAs are still in flight
                warm_in = rpool.tile([128, 128], bf16, tag="wi", name="warm_in")
                nc.vector.memset(warm_in[:], 0.0)
                warm = psq.tile([128, SB], f32, tag="acc", name="warm")
                for _ in range(40):
                    nc.tensor.matmul(warm[:, 0:64], warm_in[:],
                                     warm_in[:, 0:64], start=True, stop=True)
                warm_rd = rpool.tile([1, SB], f32, tag="wr", name="warm_rd")
                nc.vector.tensor_copy(warm_rd[0:1, 0:64], warm[0:1, 0:64])
                for _ in emit_qkv(0, wts[0], *cur, half_major=True):
                    pass
                wts[1] = prefetch_w(1)

                def attn_j(p, j, qt, kt, vs_, yn):
                    """One q-block of causal attention for pair p.

                    Score blocks are processed two at a time so the four
                    row-tiled score matmuls pipeline with full 2-head
                    concurrency; PV lags one block-pair behind.
                    """
                    if True:
                        nkb_j = 4 * (j + 1)
                        yaccs = [
                            psy.tile([128, SB], f32, tag="y", name="yacc")
                            for _ in range(2)
                        ]
                        pend = []
                        for ki in range(nkb_j // 2):
                            newp = []
                            for kb in (2 * ki, 2 * ki + 1):
                                d = kb - 4 * j
                                qlo = max(0, d) * 128  # causal column trim
                                sc = psc.tile([128, 2, SB], f32, tag="sc",
                                              name="sc")
                                for h in range(2):
                                    hlo, hhi = h * 64, (h + 1) * 64
                                    nc.tensor.matmul(
                                        sc[:, h, qlo:],
                                        kt[hlo:hhi, kb * 128:(kb + 1) * 128],
                                        qt[hlo:hhi,
                                           j * SB + qlo:(j + 1) * SB],
                                        start=True,
                                        stop=True,
                                    )
                                newp.append((kb, sc, qlo))
                            for bi, (kb, sc, qlo) in enumerate(newp):
                                d = kb - 4 * j
                                pt = ppool.tile([128, 2, SB], bf16, tag="p",
                                                name="pt")
                                nc.scalar.activation(
                                    pt[:, :, qlo:],
                                    sc[:, :, qlo:],
                                    mybir.ActivationFunctionType.Exp,
                                    scale=0.125,
                                )
                                if d >= 0:
                                    for h in range(2):
                                        nc.vector.tensor_mul(
                                            pt[:, h, qlo:],
                                            pt[:, h, qlo:],
                                            mask_sb[:, d, qlo:],
                                        )
                                newp[bi] = (kb, pt, qlo)
                            for kb_, pt_, qlo_ in pend:
                                for h in range(2):
                                    nc.tensor.matmul(
                                        yaccs[h][0:65, qlo_:],
                                        vs_[:, kb_, h, 0:65],
                                        pt_[:, h, qlo_:],
                                        start=(kb_ == 0),
                                        stop=False,
                                    )
                            pend = newp
                            pump(3 if p == NP - 1 else 2)
                        for kb_, pt_, qlo_ in pend:
                            for h in range(2):
                                nc.tensor.matmul(
                                    yaccs[h][0:65, qlo_:],
                                    vs_[:, kb_, h, 0:65],
                                    pt_[:, h, qlo_:],
                                    start=(kb_ == 0),
                                    stop=(kb_ == nkb_j - 1),
                                )
                        # ---- per-j normalize: yn = yacc[0:64] / yacc[64].
                        # The reciprocal row is broadcast across partitions
                        # by one DMA with a stride-0 free dim.
                        last = (p == NP - 1 and j == NSB - 1)
                        y_uns = []
                        rts = []
                        for h in range(2):
                            if not last:
                                y_un = yunp.tile([65, SB], f32, tag="yun",
                                                 name="y_un")
                                nc.vector.tensor_copy(y_un[:],
                                                      yaccs[h][0:65, :])
                                den = y_un[64:65, :]
                            else:
                                # final q-block: no need to free PSUM fast;
                                # read yacc directly to shorten the chain
                                y_un = yaccs[h]
                                den = yaccs[h][64:65, :]
                            rt = rpool.tile([1, SB], f32, tag="r",
                                            name="rt")
                            # custom-DVE recip only works at base
                            # partition 0: cross-partition copy first
                            nc.vector.tensor_copy(rt[:], den)
                            nc.vector.reciprocal_approx_fast(rt[:], rt[:])
                            y_uns.append(y_un)
                            rts.append(rt)
                        rbs = []
                        for h in range(2):
                            rsc = rdpool.tile([1, SB], f32, tag="rsc",
                                              name="rsc")
                            nc.sync.dma_start(rsc[:], rts[h][:])
                            rb = rpool.tile([64, SB], f32, tag="rb",
                                            name="rb")
                            nc.sync.dma_start(
                                rb[:], rsc.to_broadcast([64, SB]))
                            rbs.append(rb)
                        # (write h, read h) stay adjacent per head: a write
                        # whose recip isn't ready must not block the other
                        # head's broadcast read in the ring FIFO
                        for h in range(2):
                            hlo, hhi = h * 64, (h + 1) * 64
                            nc.vector.tensor_mul(
                                yn[hlo:hhi, j * SB:(j + 1) * SB],
                                y_uns[h][0:64, :],
                                rbs[h][:],
                            )
                        pump()

                for p in range(NP):
                    qt, kt, vt, vs_, vst_ = cur
                    if p + 2 < NP:
                        wts[p + 2] = prefetch_w(p + 2)
                    if p + 1 < NP:
                        nxt = alloc_pair(p + 1)
                        pending.append(emit_qkv(p + 1, wts[p + 1], *nxt))
                    else:
                        nxt = None
                    yn = ynormp.tile([128, S], bf16, tag="yn", name="yn")
                    ynorm.append(yn)
                    for j in range(NSB):
                        attn_j(p, j, qt, kt, vs_, yn)
                        if p == NP - 1:
                            if j == 2:
                                # hold back half of j=2's output projection:
                                # it has no dependency on the last q-block's
                                # softmax, so it fills the PE while the
                                # final normalize chain completes
                                pending.append(emit_outproj(2, (0, 1)))
                            elif j == 3:
                                pending.append(emit_outproj(2, (2, 3)))
                                pending.append(emit_outproj(3))
                            else:
                                pending.append(emit_outproj(j))
                            pump()
                    drain()
                    cur = nxt

    nc.compile()
    return nc


def _get_program():
    if "nc" not in _CACHE:
        _CACHE["nc"] = _build_program()
    return _CACHE["nc"]


def kernel(x, Wq, Wk, Wv, Wo, bo):
    global LAST_RESULTS
    x = np.asarray(x, dtype=np.float32)
    Wq = np.asarray(Wq, dtype=np.float32)
    Wk = np.asarray(Wk, dtype=np.float32)
    Wv = np.asarray(Wv, dtype=np.float32)
    Wo = np.asarray(Wo, dtype=np.float32)
    bo = np.asarray(bo, dtype=np.float32)

    nc = _get_program()

    # constants shared by all cores
    masks = np.zeros((128, 4, SB), dtype=ml_dtypes.bfloat16)
    kk = np.arange(128)[:, None]
    qq = np.arange(SB)[None, :]
    for d in range(4):
        masks[:, d, :] = (128 * d + kk <= qq).astype(ml_dtypes.bfloat16)
    bo2 = np.ascontiguousarray((bo / 2.0).reshape(NCH, 128).T)

    def wshape(w):  # [D, DG] -> [NP, 128, NCH, 128] bf16
        return np.ascontiguousarray(
            w.reshape(NCH, 128, NP, 128).transpose(2, 1, 0, 3)
        ).astype(ml_dtypes.bfloat16)

    in_maps = []
    for c in range(8):
        b, g = c // 2, c % 2
        xT = np.ascontiguousarray(x[b].T).astype(ml_dtypes.bfloat16)
        wo_g = Wo[g * DG:(g + 1) * DG, :]
        wo_dev = np.ascontiguousarray(
            wo_g.reshape(NP, 128, NCH, 128).transpose(1, 0, 2, 3)
        ).astype(ml_dtypes.bfloat16)
        in_maps.append({
            "xT": xT,
            "Wq": wshape(Wq[:, g * DG:(g + 1) * DG]),
            "Wk": wshape(Wk[:, g * DG:(g + 1) * DG]),
            "Wv": wshape(Wv[:, g * DG:(g + 1) * DG]),
            "Wo": wo_dev,
            "bo2": bo2,
            "masks": masks,
        })

    trace = bool(os.environ.get("BASS_TRACE"))
    if trace:
        _install_trace_shim()
    res = None
    for attempt in range(3):
        try:
            res = bass_utils.run_bass_kernel_spmd(
                nc, in_maps, core_ids=list(range(8)), trace=trace)
            break
        except Exception:
            if attempt == 2:
                raise
    LAST_RESULTS = res

    out = np.empty((B, S, D), dtype=np.float32)
    for b in range(B):
        acc = (res.results[2 * b]["outT"].astype(np.float32)
               + res.results[2 * b + 1]["outT"].astype(np.float32))
        out[b] = acc.T
    return out


# revision 32
# speedup vs baseline: 1.0397x; 1.0241x over previous
"""Multi-head causal attention (B=4, S=2048, D=1024, H=16) on 8 TRN2 cores.

Sharding: core c = (batch b = c//2, head-group g = c%2). Each core computes
8 heads of one batch end-to-end: QKV projections, causal flash attention,
and its half of the output projection (row-parallel Wo). Host sums the two
partial outputs per batch (the "all-reduce"); bias is added on device,
split half per core. Output partials are bf16 (summed in fp32 on host).

Device dataflow is fully transposed (xT in, outT out) so no on-device
transposes of activations are needed; V is transposed via one xbar DMA
transpose per pair (into a partition-tiled [128, NKB, 128] layout) plus
cheap DVE re-copies that insert the denominator ones column. All matmuls
are bf16 (fp32 PSUM accumulation) except a tiny f32r matmul that
broadcasts softmax reciprocals across partitions. Scores for the two
heads of a pair are issued back-to-back into disjoint PE row groups so
they run concurrently (contraction is only 64 deep). The causal structure
skips invalid 128x512 blocks entirely and trims the invalid left columns
of diagonal blocks from the scores/exp/mask/PV chain.

Scheduling: the next pair's QKV projection matmuls (and, on the last
pair, the output-projection matmuls) are pumped into the Tensor queue
between attention blocks so the PE stays busy while the Scalar engine
works through the exp chain. Softmax normalization happens per q-block
(reading the PV accumulator directly from PSUM) instead of per pair.
Input/weight DMAs are split across both HWDGE rings (sync + scalar) and
ordered so the first matmul can start as early as possible.
"""
import os
import sys
import types

import numpy as np
import ml_dtypes

from concourse import bacc, tile, bass_utils, mybir

B, S, D, H = 4, 2048, 1024, 16
HD = 64            # head dim
G = 2              # head groups (cores per batch)
DG = D // G        # 512 cols per core
NP = DG // 128     # 4 head-pairs per core
NCH = D // 128     # 8 contraction chunks
SB = 512           # q block
NSB = S // SB      # 4 q blocks
NKB = S // 128     # 16 k blocks

f32 = mybir.dt.float32
f32r = mybir.dt.float32r
bf16 = mybir.dt.bfloat16

LAST_RESULTS = None
_CACHE = {}


def _install_trace_shim():
    """Register the axon NTFF profile hook if this image's antenv lacks it."""
    if "antenv.axon_hooks" in sys.modules:
        return
    try:
        from trn_agent_boot.trn_boot import _ntff_profile_via_ctypes

        hook = _ntff_profile_via_ctypes("/opt/axon/libaxon_pjrt.so")
        mod = types.ModuleType("antenv.axon_hooks")
        mod.get_axon_ntff_profile_hook = lambda: hook
        mod.set_axon_ntff_profile_hook = lambda h: None
        sys.modules["antenv.axon_hooks"] = mod
        import antenv

        antenv.axon_hooks = mod
    except Exception:
        pass


def _build_program():
    nc = bacc.Bacc("TRN2", target_bir_lowering=False, debug=False)

    xT_d = nc.dram_tensor("xT", [D, S], bf16, kind="ExternalInput").ap()
    wq_d = nc.dram_tensor("Wq", [NP, 128, NCH, 128], bf16, kind="ExternalInput").ap()
    wk_d = nc.dram_tensor("Wk", [NP, 128, NCH, 128], bf16, kind="ExternalInput").ap()
    wv_d = nc.dram_tensor("Wv", [NP, 128, NCH, 128], bf16, kind="ExternalInput").ap()
    wo_d = nc.dram_tensor("Wo", [128, NP, NCH, 128], bf16, kind="ExternalInput").ap()
    bo_d = nc.dram_tensor("bo2", [128, NCH], f32, kind="ExternalInput").ap()
    mask_d = nc.dram_tensor("masks", [128, 4, SB], bf16, kind="ExternalInput").ap()
    out_d = nc.dram_tensor("outT", [D, S], bf16, kind="ExternalOutput").ap()

    xT_src = xT_d.rearrange("(c k) s -> k c s", k=128)
    out_dst = out_d.rearrange("(c k) s -> k c s", k=128)

    with tile.TileContext(nc) as tc:
        with (
            tc.tile_pool(name="const", bufs=1) as constp,
            tc.tile_pool(name="psc", bufs=2, space="PSUM") as psc,
            tc.tile_pool(name="psq", bufs=2, space="PSUM") as psq,
            tc.tile_pool(name="psy", bufs=2, space="PSUM") as psy,
            tc.tile_pool(name="ynormp", bufs=4) as ynormp,
            tc.tile_pool(name="outp", bufs=3) as outp,
        ):
            mask_sb = constp.tile([128, 4, SB], bf16)
            bo_sb = constp.tile([128, NCH], f32)
            wo_sb = constp.tile([128, NP, NCH, 128], bf16)
            # scalar HWDGE ring: tiny consts now; masks/wo are issued
            # after the xt quarters so they never delay the first matmuls.
            nc.scalar.dma_start(bo_sb[:], bo_d[:])

            ynorm = []  # per-pair [128, S] bf16 normalized attention outputs

            with (
                tc.tile_pool(name="xtp", bufs=1) as xtp,
                tc.tile_pool(name="wp", bufs=3) as wp,
                tc.tile_pool(name="qkv", bufs=2) as qkvp,
                tc.tile_pool(name="vtp", bufs=2) as vtp,
                tc.tile_pool(name="vstp", bufs=2) as vstp,
                tc.tile_pool(name="vp", bufs=2) as vpool,
                tc.tile_pool(name="pp", bufs=4) as ppool,
                tc.tile_pool(name="yun", bufs=4) as yunp,
                tc.tile_pool(name="rp", bufs=2) as rpool,
                tc.tile_pool(name="rdp", bufs=4, space="DRAM") as rdpool,
            ):
                xt = xtp.tile([128, NCH, S], bf16)

                def alloc_pair(pp_):
                    qt_ = qkvp.tile([128, S], bf16, tag="qt", name="qt")
                    kt_ = qkvp.tile([128, S], bf16, tag="kt", name="kt")
                    vt_ = vtp.tile([128, S], bf16, tag="vt", name="vt")
                    # per (kb, h): 64 head dims + ones col at 64 (+1 pad
                    # so the h stride is 4B-aligned for fast DVE copies)
                    vs_ = vpool.tile([128, NKB, 2, 66], bf16, tag="v",
                                     name="v_sb")
                    vst_ = vstp.tile([128, NKB, 128], bf16, tag="vst",
                                     name="vst")
                    nc.vector.memset(vs_[:, :, :, 64:65], 1.0)
                    return qt_, kt_, vt_, vs_, vst_

                def prefetch_w(pp_):
                    tiles = []
                    for nm, wd in (("wv", wv_d), ("wq", wq_d), ("wk", wk_d)):
                        wt = wp.tile([128, NCH, 128], bf16, tag=nm, name=nm)
                        nc.sync.dma_start(wt[:], wd[pp_])
                        tiles.append(wt)
                    return tiles

                def emit_qkv(pp_, w_tiles, qt_, kt_, vt_, vs_, vst_,
                             half_major=False):
                    """Yield small units of pair pp_'s QKV projection work.

                    V is projected first so its transpose (one xbar DMA
                    for the whole [128, S] tile) can start early. Weight
                    tiles were DMA'd well in advance by prefetch_w.
                    half_major=True consumes the input in S-halves (for
                    the prologue pair, paced by the xt quarter DMAs).
                    """
                    yield
                    halves = ((0, 1),) if not half_major else ((0,), (1,))
                    for hgrp in halves:
                      for half in hgrp:
                        for wi, (wt, dst) in enumerate(
                                zip(w_tiles, (vt_, qt_, kt_))):
                            if half == 1 and not half_major:
                                continue
                            for sblk in ((2 * half, 2 * half + 1)
                                         if half_major else (0, 1, 2, 3)):
                                acc = psq.tile([128, SB], f32, tag="acc",
                                               name="qacc")
                                for ci in range(NCH):
                                    nc.tensor.matmul(
                                        acc[:],
                                        wt[:, ci, :],
                                        xt[:, ci,
                                           sblk * SB:(sblk + 1) * SB],
                                        start=(ci == 0),
                                        stop=(ci == NCH - 1),
                                    )
                                    if ci % 4 == 3:
                                        yield
                                nc.vector.tensor_copy(
                                    dst[:, sblk * SB:(sblk + 1) * SB],
                                    acc[:],
                                )
                                yield
                            if wi == 0 and (half == 1 or not half_major):
                                # vt complete: one xbar transpose VT -> V
                                # (s = kb*128 + partition tiling), then
                                # per-kb DVE copies inserting the ones col.
                                nc.sync.dma_start_transpose(vst_[:], vt_[:])
                                for kb in range(NKB):
                                    nc.vector.tensor_copy(
                                        vs_[:, kb, :, 0:64],
                                        vst_[:, kb, :].rearrange(
                                            "k (h d) -> k h d", h=2),
                                    )
                                    if kb % 4 == 3:
                                        yield

                def emit_outproj(j, g2s=(0, 1, 2, 3)):
                    """outT[:, j block] = Wo_g.T @ ynorm (+ bo/2), bf16."""
                    for g2 in g2s:
                        ot = outp.tile([128, 2, SB], bf16, tag="ot",
                                       name="ot")
                        for si in range(2):
                            dc = 2 * g2 + si
                            acc = psq.tile([128, SB], f32, tag="acc",
                                           name="oacc")
                            for pp in range(NP):
                                nc.tensor.matmul(
                                    acc[:],
                                    wo_sb[:, pp, dc, :],
                                    ynorm[pp][:, j * SB:(j + 1) * SB],
                                    start=(pp == 0),
                                    stop=(pp == NP - 1),
                                )
                                if pp % 2 == 1:
                                    yield
                            nc.vector.tensor_scalar_add(
                                ot[:, si, :],
                                acc[:],
                                bo_sb[:, dc:dc + 1],
                            )
                        nc.sync.dma_start(
                            out_dst[:, 2 * g2:2 * g2 + 2,
                                    j * SB:(j + 1) * SB],
                            ot[:],
                        )
                        yield

                pending = []

                def pump(n=1):
                    for _ in range(n):
                        while pending:
                            try:
                                next(pending[0])
                                break
                            except StopIteration:
                                pending.pop(0)
                        else:
                            break

                def drain():
                    while pending:
                        try:
                            next(pending[0])
                        except StopIteration:
                            pending.pop(0)

                # prologue: pair-0 weights on the sync ring first, then x
                wts = {0: prefetch_w(0)}
                cur = alloc_pair(0)
                for qtr in range(4):
                    nc.sync.dma_start(
                        xt[:, :, qtr * SB:(qtr + 1) * SB],
                        xT_src[:, :, qtr * SB:(qtr + 1) * SB],
                    )
                nc.sync.dma_start(mask_sb[:], mask_d[:])
                nc.sync.dma_start(wo_sb[:], wo_d[:])
                # HAM warmup: ~4us of dummy matmuls un-throttle the PE clock
                # (1.2 -> 2.4 GHz) while the input DMAs are still in flight
                warm_in = rpool.tile([128, 128], bf16, tag="wi", name="warm_in")
                nc.vector.memset(warm_in[:], 0.0)
                warm = psq.tile([128, SB], f32, tag="acc", name="warm")
                for _ in range(40):
                    nc.tensor.matmul(warm[:, 0:64], warm_in[:],
                                     warm_in[:, 0:64], start=True, stop=True)
                warm_rd = rpool.tile([1, SB], f32, tag="wr", name="warm_rd")
                nc.vector.tensor_copy(warm_rd[0:1, 0:64], warm[0:1, 0:64])
                for _ in emit_qkv(0, wts[0], *cur, half_major=True):
                    pass
                wts[1] = prefetch_w(1)

                def attn_j(p, j, qt, kt, vs_, yn):
                    """One q-block of causal attention for pair p.

                    Score blocks are processed two at a time so the four
                    row-tiled score matmuls pipeline with full 2-head
                    concurrency; PV lags one block-pair behind.
                    """
                    if True:
                        nkb_j = 4 * (j + 1)
                        yaccs = [
                            psy.tile([128, SB], f32, tag="y", name="yacc")
                            for _ in range(2)
                        ]
                        pend = []
                        for ki in range(nkb_j // 2):
                            newp = []
                            for kb in (2 * ki, 2 * ki + 1):
                                d = kb - 4 * j
                                qlo = max(0, d) * 128  # causal column trim
                                sc = psc.tile([128, 2, SB], f32, tag="sc",
                                              name="sc")
                                for h in range(2):
                                    hlo, hhi = h * 64, (h + 1) * 64
                                    nc.tensor.matmul(
                                        sc[:, h, qlo:],
                                        kt[hlo:hhi, kb * 128:(kb + 1) * 128],
                                        qt[hlo:hhi,
                                           j * SB + qlo:(j + 1) * SB],
                                        start=True,
                                        stop=True,
                                    )
                                newp.append((kb, sc, qlo))
                            for bi, (kb, sc, qlo) in enumerate(newp):
                                d = kb - 4 * j
                                pt = ppool.tile([128, 2, SB], bf16, tag="p",
                                                name="pt")
                                nc.scalar.activation(
                                    pt[:, :, qlo:],
                                    sc[:, :, qlo:],
                                    mybir.ActivationFunctionType.Exp,
                                    scale=0.125,
                                )
                                if d >= 0:
                                    for h in range(2):
                                        nc.vector.tensor_mul(
                                            pt[:, h, qlo:],
                                            pt[:, h, qlo:],
                                            mask_sb[:, d, qlo:],
                                        )
                                newp[bi] = (kb, pt, qlo)
                            for kb_, pt_, qlo_ in pend:
                                for h in range(2):
                                    nc.tensor.matmul(
                                        yaccs[h][0:65, qlo_:],
                                        vs_[:, kb_, h, 0:65],
                                        pt_[:, h, qlo_:],
                                        start=(kb_ == 0),
                                        stop=False,
                                    )
                            pend = newp
                            pump(3 if p == NP - 1 else 2)
                        for kb_, pt_, qlo_ in pend:
                            for h in range(2):
                                nc.tensor.matmul(
                                    yaccs[h][0:65, qlo_:],
                                    vs_[:, kb_, h, 0:65],
                                    pt_[:, h, qlo_:],
                                    start=(kb_ == 0),
                                    stop=(kb_ == nkb_j - 1),
                                )
                        # ---- per-j normalize: yn = yacc[0:64] / yacc[64].
                        # The reciprocal row is broadcast across partitions
                        # by one DMA with a stride-0 free dim.
                        last = (p == NP - 1 and j == NSB - 1)
                        y_uns = []
                        rts = []
                        for h in range(2):
                            if not last:
                                y_un = yunp.tile([65, SB], f32, tag="yun",
                                                 name="y_un")
                                nc.vector.tensor_copy(y_un[:],
                                                      yaccs[h][0:65, :])
                                den = y_un[64:65, :]
                            else:
                                # final q-block: no need to free PSUM fast;
                                # read yacc directly to shorten the chain
                                y_un = yaccs[h]
                                den = yaccs[h][64:65, :]
                            rt = rpool.tile([1, SB], f32, tag="r",
                                            name="rt")
                            # custom-DVE recip only works at base
                            # partition 0: cross-partition copy first
                            nc.vector.tensor_copy(rt[:], den)
                            nc.vector.reciprocal_approx_fast(rt[:], rt[:])
                            y_uns.append(y_un)
                            rts.append(rt)
                        rbs = []
                        for h in range(2):
                            rsc = rdpool.tile([1, SB], f32, tag="rsc",
                                              name="rsc")
                            nc.sync.dma_start(rsc[:], rts[h][:])
                            rb = rpool.tile([64, SB], f32, tag="rb",
                                            name="rb")
                            nc.sync.dma_start(
                                rb[:], rsc.to_broadcast([64, SB]))
                            rbs.append(rb)
                        # (write h, read h) stay adjacent per head: a write
                        # whose recip isn't ready must not block the other
                        # head's broadcast read in the ring FIFO
                        for h in range(2):
                            hlo, hhi = h * 64, (h + 1) * 64
                            nc.vector.tensor_mul(
                                yn[hlo:hhi, j * SB:(j + 1) * SB],
                                y_uns[h][0:64, :],
                                rbs[h][:],
                            )
                        pump()

                for p in range(NP):
                    qt, kt, vt, vs_, vst_ = cur
                    if p + 2 < NP:
                        wts[p + 2] = prefetch_w(p + 2)
                    if p + 1 < NP:
                        nxt = alloc_pair(p + 1)
                        pending.append(emit_qkv(p + 1, wts[p + 1], *nxt))
                    else:
                        nxt = None
                    yn = ynormp.tile([128, S], bf16, tag="yn", name="yn")
                    ynorm.append(yn)
                    for j in range(NSB):
                        attn_j(p, j, qt, kt, vs_, yn)
                        if p == NP - 1:
                            if j == 1:
                                pending.append(emit_outproj(1, (0, 1)))
                            elif j == 2:
                                # hold back half of j=2's output projection:
                                # it has no dependency on the last q-block's
                                # softmax, so it fills the PE while the
                                # final normalize chain completes
                                pending.append(emit_outproj(2, (0, 1)))
                            elif j == 3:
                                pending.append(emit_outproj(1, (2, 3)))
                                pending.append(emit_outproj(2, (2, 3)))
                                pending.append(emit_outproj(3))
                            else:
                                pending.append(emit_outproj(j))
                            pump()
                    drain()
                    cur = nxt

    nc.compile()
    return nc


def _get_program():
    if "nc" not in _CACHE:
        _CACHE["nc"] = _build_program()
    return _CACHE["nc"]


def kernel(x, Wq, Wk, Wv, Wo, bo):
    global LAST_RESULTS
    x = np.asarray(x, dtype=np.float32)
    Wq = np.asarray(Wq, dtype=np.float32)
    Wk = np.asarray(Wk, dtype=np.float32)
    Wv = np.asarray(Wv, dtype=np.float32)
    Wo = np.asarray(Wo, dtype=np.float32)
    bo = np.asarray(bo, dtype=np.float32)

    nc = _get_program()

    # constants shared by all cores
    masks = np.zeros((128, 4, SB), dtype=ml_dtypes.bfloat16)
    kk = np.arange(128)[:, None]
    qq = np.arange(SB)[None, :]
    for d in range(4):
        masks[:, d, :] = (128 * d + kk <= qq).astype(ml_dtypes.bfloat16)
    bo2 = np.ascontiguousarray((bo / 2.0).reshape(NCH, 128).T)

    def wshape(w):  # [D, DG] -> [NP, 128, NCH, 128] bf16
        return np.ascontiguousarray(
            w.reshape(NCH, 128, NP, 128).transpose(2, 1, 0, 3)
        ).astype(ml_dtypes.bfloat16)

    in_maps = []
    for c in range(8):
        b, g = c // 2, c % 2
        xT = np.ascontiguousarray(x[b].T).astype(ml_dtypes.bfloat16)
        wo_g = Wo[g * DG:(g + 1) * DG, :]
        wo_dev = np.ascontiguousarray(
            wo_g.reshape(NP, 128, NCH, 128).transpose(1, 0, 2, 3)
        ).astype(ml_dtypes.bfloat16)
        in_maps.append({
            "xT": xT,
            "Wq": wshape(Wq[:, g * DG:(g + 1) * DG]),
            "Wk": wshape(Wk[:, g * DG:(g + 1) * DG]),
            "Wv": wshape(Wv[:, g * DG:(g + 1) * DG]),
            "Wo": wo_dev,
            "bo2": bo2,
            "masks": masks,
        })

    trace = bool(os.environ.get("BASS_TRACE"))
    if trace:
        _install_trace_shim()
    res = None
    for attempt in range(3):
        try:
            res = bass_utils.run_bass_kernel_spmd(
                nc, in_maps, core_ids=list(range(8)), trace=trace)
            break
        except Exception:
            if attempt == 2:
                raise
    LAST_RESULTS = res

    out = np.empty((B, S, D), dtype=np.float32)
    for b in range(B):
        acc = (res.results[2 * b]["outT"].astype(np.float32)
               + res.results[2 * b + 1]["outT"].astype(np.float32))
        out[b] = acc.T
    return out
